# revision 29
# baseline (speedup 1.0000x reference)
"""CCA-SSG GNN (2-layer GraphConv encoder x2 graphs + per-feature z-score) on 8 TRN2 cores.

Strategy (dst-sharded graph parallel):
  - Nodes are partitioned across 8 cores (12500 each, by node-id range).
  - Per core, nodes are packed into NBLK blocks of 128 "slots" such that each
    (block, src-bucket) cell holds <= CELL edges (balanced greedy packing).
  - The SpMM (segment_sum of gathered rows) runs per 128-slot block: edge rows
    are fetched as 128-lane tiles via dma_gather (HBM row gather, 4 SWDGE
    queues) and reduced on the tensor engine against 0/1 one-hot matrices
    BUILT ON-CHIP (DVE is_equal of a resident slot-id table vs an iota row):
    psum[feat, slot] += sum_lane gathered[lane, feat] * OH01[lane, slot].
  - Degree norms are factored out of the one-hot: s_out[src] is pre-folded
    into the gathered tables (host-prescaled feat for layer 1; device-folded
    into h1 rows for layer 2), and s_in[dst] is applied per destination slot
    via the scalar engine's per-partition activation scale. The GraphConv
    bias enters as a rank-1 (K=1) accumulation matmul so a single activation
    computes relu/identity(alpha * (aggW + r*b)).
  - W is applied AFTER aggregation (linearity), so layer-1 gathers prescaled
    bf16 feat rows directly. Layer-2 gathers h1 rows from an AllGather'ed
    bf16 [8*NBLK*128, 128] table (permuted node order).
  - int16 gather indices limit the source window to <32768 rows, so sources
    are split into 4 buckets; each (group-of-blocks, bucket) run is one
    dma_gather.
  - z-score: per-block ones-vector matmuls accumulate sum/sumsq (padding
    slots are exact zeros), tiny AllGather + on-chip reduction, then
    (h2 - mean) * rsqrt(var) per column.
"""

import os
import sys
import types
import numpy as np
import ml_dtypes  # noqa: F401  (registers bfloat16)

P = 128
NC = 8
HDT = np.dtype("float16")

_ENV_READY = False


def _ensure_env():
    global _ENV_READY
    if _ENV_READY:
        return
    if "/root/.axon_site" not in sys.path and os.path.isdir("/root/.axon_site"):
        sys.path.insert(0, "/root/.axon_site")
    # NTFF profiling hook (missing antenv.axon_hooks in container): degrade to
    # no-trace silently if unavailable.
    if "antenv.axon_hooks" not in sys.modules:
        try:
            from trn_agent_boot.trn_boot import _ntff_profile_via_ctypes
            mod = types.ModuleType("antenv.axon_hooks")
            hook = _ntff_profile_via_ctypes("/opt/axon/libaxon_pjrt.so")
            mod.get_axon_ntff_profile_hook = lambda: hook
            mod.set_axon_ntff_profile_hook = lambda h: None
            sys.modules["antenv.axon_hooks"] = mod
        except Exception:
            pass
    _ENV_READY = True


class Cfg:
    def __init__(self, n=100000, ngrp=8, nblk=104, cell=256):
        assert nblk % ngrp == 0
        self.n = n
        self.npc = n // NC               # nodes per core
        self.nbuck = 4
        self.l1w = n // self.nbuck       # layer-1 src bucket window
        self.ngrp = ngrp
        self.nblk = nblk
        self.cell = cell                 # lanes per (block, bucket) cell
        self.grp_sz = nblk // ngrp
        self.nodes_pad = nblk * P
        self.npad = NC * self.nodes_pad
        self.l2w = 2 * self.nodes_pad    # layer-2 bucket window (2 cores)
        self.lanes = nblk * self.nbuck * cell
        self.tiles = self.lanes // P
        self.tpc = cell // P             # tiles per cell
        self.run_tiles = self.grp_sz * self.tpc
        assert self.l1w <= 32767 and self.l2w <= 32767
        assert self.npc <= nblk * P


# ---------------------------------------------------------------------------
# Host-side graph preprocessing
# ---------------------------------------------------------------------------

def _pack_blocks(coords, cfg):
    """Greedy balanced packing of nodes into blocks, vectorized across cores.

    coords: [NC, npc, nbuck] int32 per-(core-local node) bucket in-degree.
    Returns assign [NC, npc] block id, or None if infeasible.
    """
    nblk, cellcap = cfg.nblk, cfg.cell
    tot = coords.sum(2)
    order = np.argsort(-tot, axis=1, kind="stable")
    loads = np.zeros((NC, nblk, cfg.nbuck), np.int32)
    counts = np.zeros((NC, nblk), np.int32)
    assign = np.full((NC, cfg.npc), -1, np.int32)
    ar = np.arange(NC)
    for r in range(cfg.npc):
        nid = order[:, r]
        c = coords[ar, nid]                       # [NC, nbuck]
        cand = loads + c[:, None, :]              # [NC, nblk, nbuck]
        feas = (cand <= cellcap).all(2) & (counts < P)
        if not feas.any(1).all():
            return None
        score = cand.max(2).astype(np.float32) + counts.astype(np.float32) * 1e-3
        score[~feas] = np.inf
        pick = score.argmin(1)
        assign[ar, nid] = pick
        loads[ar, pick] += c
        counts[ar, pick] += 1
    return assign


def _prep_graph(src, dst, cfg):
    """Build per-core device arrays for one graph. Returns dict or None (repack)."""
    n, npc, nblk = cfg.n, cfg.npc, cfg.nblk

    deg_out = np.bincount(src, minlength=n).astype(np.float32)
    deg_in = np.bincount(dst, minlength=n).astype(np.float32)
    s_out = np.maximum(deg_out, 1.0) ** -0.5
    s_in = np.maximum(deg_in, 1.0) ** -0.5

    bkt = src // cfg.l1w                              # [E] edge bucket
    coords_flat = np.bincount(dst * cfg.nbuck + bkt, minlength=n * cfg.nbuck)
    coords = coords_flat.reshape(n, cfg.nbuck).astype(np.int32)
    coords_by_core = coords.reshape(NC, npc, cfg.nbuck)

    assign = _pack_blocks(coords_by_core, cfg)
    if assign is None:
        return None

    # slot within block (stable by local node id), global padded id
    perm_row = np.empty(n, np.int64)                 # node -> block*128 + slot
    blk_counts = np.zeros((NC, nblk), np.int32)
    for k in range(NC):
        a = assign[k]
        sidx = np.argsort(a, kind="stable")
        cnt = np.bincount(a, minlength=nblk)
        starts = np.concatenate([[0], np.cumsum(cnt)[:-1]])
        within = np.arange(npc) - np.repeat(starts, cnt)
        rows = a[sidx] * P + within
        pr = np.empty(npc, np.int64)
        pr[sidx] = rows
        perm_row[k * npc:(k + 1) * npc] = pr
        blk_counts[k] = cnt
    pid = (np.arange(n) // npc) * cfg.nodes_pad + perm_row  # node -> padded row

    cores = {}
    for k in range(NC):
        m = (dst // npc) == k
        es, ed, eb = src[m], dst[m], bkt[m]
        eblk = perm_row[ed] // P
        eslot = perm_row[ed] % P
        # run index in the lane stream: (group, bucket, cell-in-group)
        run = ((eblk // cfg.grp_sz) * cfg.nbuck + eb) * cfg.grp_sz + (eblk % cfg.grp_sz)
        o = np.lexsort((es, run))
        es, eslot, run, eb = es[o], eslot[o], run[o], eb[o]
        cnt = np.bincount(run, minlength=nblk * cfg.nbuck)
        if cnt.max() > cfg.cell:
            return None
        starts = np.concatenate([[0], np.cumsum(cnt)[:-1]])
        within = np.arange(len(run)) - np.repeat(starts, cnt)
        lane = run * cfg.cell + within

        idx1 = np.zeros(cfg.lanes, np.int16)
        idx2 = np.zeros(cfg.lanes, np.int16)
        idx1[lane] = (es - eb * cfg.l1w).astype(np.int16)
        idx2[lane] = (pid[es] - eb.astype(np.int64) * cfg.l2w).astype(np.int16)
        # slot-id table for on-chip one-hot build: [lane%128, tile], -1 = empty
        sid = np.full((P, cfg.tiles), -1.0, HDT)
        sid[lane % P, lane // P] = eslot.astype(HDT)

        idxA = np.tile(idx1.reshape(-1, 16).T, (8, 1)).astype(np.int16)
        idxB = np.tile(idx2.reshape(-1, 16).T, (8, 1)).astype(np.int16)

        # per-slot scale tables [P, nblk]: alpha1 = s_out*s_in (layer-1 out,
        # with next layer's s_out prefold), alpha2 = s_in, rinv = 1/s_in
        # (rank-1 bias coefficient). Zero on padding slots -> exact h = 0.
        nodes = np.arange(k * npc, (k + 1) * npc)
        a1 = np.zeros((P, nblk), np.float32)
        a2 = np.zeros((P, nblk), np.float32)
        rv = np.zeros((P, nblk), np.float32)
        pslot = perm_row[nodes] % P
        pblk = perm_row[nodes] // P
        a1[pslot, pblk] = (s_out[nodes] * s_in[nodes]).astype(np.float32)
        a2[pslot, pblk] = s_in[nodes].astype(np.float32)
        rv[pslot, pblk] = (1.0 / s_in[nodes]).astype(np.float32)
        # rinv used as K=1 matmul lhsT: [1, nblk*P] (slot-major per block)
        rinv = np.ascontiguousarray(rv.T.reshape(1, nblk * P)).astype(HDT)
        cores[k] = dict(idxA=idxA, idxB=idxB, sid=sid,
                        alpha1=a1, alpha2=a2, rinv=rinv)

    rowmap = (np.arange(n) // npc) * cfg.nodes_pad + perm_row
    return dict(cores=cores, rowmap=rowmap, s_out=s_out)


# ---------------------------------------------------------------------------
# Device program
# ---------------------------------------------------------------------------

def _ceil_banks(grp_sz):
    return (grp_sz + 3) // 4


def _build_program(cfg):
    import concourse.bacc as bacc
    import concourse.mybir as mybir
    from concourse.tile import TileContext

    nblk, grp_sz, tpc = cfg.nblk, cfg.grp_sz, cfg.tpc
    f32 = mybir.dt.float32
    bf16 = mybir.dt.float16
    AF = mybir.ActivationFunctionType
    OP = mybir.AluOpType

    NQ = 4
    nc = bacc.Bacc("TRN2", num_devices=NC, target_bir_lowering=False,
                   num_swdge_queues=NQ, dynamic_dma_scratch_size=32768)

    feat_t = [nc.dram_tensor(f"feat{g+1}", [cfg.n, P], bf16, kind="ExternalInput")
              for g in range(2)]
    W_t = [nc.dram_tensor(f"w{l+1}", [P, P], bf16, kind="ExternalInput")
           for l in range(2)]
    b_t = [nc.dram_tensor(f"b{l+1}", [1, P], f32, kind="ExternalInput")
           for l in range(2)]
    idxA_t = [nc.dram_tensor(f"idxa{g+1}", [P, cfg.lanes // 16], mybir.dt.int16,
                             kind="ExternalInput") for g in range(2)]
    idxB_t = [nc.dram_tensor(f"idxb{g+1}", [P, cfg.lanes // 16], mybir.dt.int16,
                             kind="ExternalInput") for g in range(2)]
    sid_t = [nc.dram_tensor(f"sid{g+1}", [P, cfg.tiles], bf16,
                            kind="ExternalInput") for g in range(2)]
    iota_t = nc.dram_tensor("iota", [P, P], bf16, kind="ExternalInput")
    al1_t = [nc.dram_tensor(f"al1{g+1}", [P, nblk], f32, kind="ExternalInput")
             for g in range(2)]
    al2_t = [nc.dram_tensor(f"al2{g+1}", [P, nblk], f32, kind="ExternalInput")
             for g in range(2)]
    rinv_t = [nc.dram_tensor(f"rinv{g+1}", [1, nblk * P], bf16,
                             kind="ExternalInput") for g in range(2)]
    z_t = [nc.dram_tensor(f"z{g+1}", [cfg.nodes_pad, P], bf16, kind="ExternalOutput")
           for g in range(2)]

    h1loc = [nc.dram_tensor(f"h1loc{g}", [cfg.nodes_pad, P], bf16) for g in range(2)]
    h1full = [nc.dram_tensor(f"h1full{g}", [cfg.npad, P], bf16, addr_space="Shared")
              for g in range(2)]
    stloc = [nc.dram_tensor(f"stloc{g}", [1, 2 * P], f32) for g in range(2)]
    stall = [nc.dram_tensor(f"stall{g}", [NC, 2 * P], f32, addr_space="Shared")
             for g in range(2)]

    qctr = [0]

    def next_q():
        qctr[0] += 1
        return qctr[0] % NQ

    with TileContext(nc) as tc:
        with tc.tile_pool(name="const", bufs=1) as cpool, \
             tc.tile_pool(name="meta", bufs=2) as mpool, \
             tc.tile_pool(name="ohp", bufs=3) as ohpool, \
             tc.tile_pool(name="gb", bufs=6) as gpool, \
             tc.tile_pool(name="rp", bufs=2) as rpool, \
             tc.tile_pool(name="work", bufs=4) as wpool, \
             tc.tile_pool(name="hg", bufs=2) as hgpool, \
             tc.tile_pool(name="big", bufs=2) as bigpool, \
             tc.tile_pool(name="pagg", bufs=_ceil_banks(grp_sz) + 1, space="PSUM") as aggpool, \
             tc.tile_pool(name="ph", bufs=2, space="PSUM") as hpool, \
             tc.tile_pool(name="pst", bufs=1, space="PSUM") as stpool:

            # --- prefetch pass-0 idx before everything else on the DMA queue
            idx0_sb = mpool.tile([P, cfg.lanes // 16], mybir.dt.int16, tag="idx",
                                 name="idxsb")
            nc.sync.dma_start(idx0_sb[:], idxA_t[0][:])

            # --- constants ---
            ones = cpool.tile([P, P], f32, tag="ones")
            nc.vector.memset(ones[:], 1.0)
            ones_bf = cpool.tile([P, P], bf16, tag="onesbf")
            nc.vector.memset(ones_bf[:], 1.0)
            iota_sb = cpool.tile([P, P], bf16, tag="iota")
            nc.sync.dma_start(iota_sb[:], iota_t[:])

            W_sb, brow_sb = [], []
            for l in range(2):
                w = cpool.tile([P, P], bf16, tag=f"w{l}", name=f"wsb{l}")
                nc.sync.dma_start(w[:], W_t[l][:])
                W_sb.append(w)
                brow = cpool.tile([1, P], f32, tag=f"browf{l}", name=f"browf{l}")
                nc.sync.dma_start(brow[:], b_t[l][:])
                brow_bf = cpool.tile([1, P], bf16, tag=f"brow{l}", name=f"brow{l}")
                nc.vector.tensor_copy(brow_bf[:], brow[:])
                brow_sb.append(brow_bf)

            sid_sb, al1_sb, al2_sb = [], [], []
            for g in range(2):
                s = cpool.tile([P, cfg.tiles], bf16, tag=f"sid{g}", name=f"sid{g}")
                nc.sync.dma_start(s[:], sid_t[g][:])
                sid_sb.append(s)
                a1 = cpool.tile([P, nblk], f32, tag=f"al1{g}", name=f"al1{g}")
                nc.sync.dma_start(a1[:], al1_t[g][:])
                al1_sb.append(a1)
                a2 = cpool.tile([P, nblk], f32, tag=f"al2{g}", name=f"al2{g}")
                nc.sync.dma_start(a2[:], al2_t[g][:])
                al2_sb.append(a2)

            h2all = [None, None]
            st_psum = [None, None]

            def load_idx(g, layer):
                idx_dram = (idxA_t if layer == 0 else idxB_t)[g]
                idx_sb = mpool.tile([P, cfg.lanes // 16], mybir.dt.int16, tag="idx",
                                    name="idxsb")
                nc.sync.dma_start(idx_sb[:], idx_dram[:])
                return idx_sb

            def gather_layer(g, layer, src_dram, window, idx_sb=None,
                             hooks=None):
                """One gather+aggregate layer for graph g. Produces per-block
                h into h1loc dram (layer 0) or h2all sbuf + stats (layer 1).
                hooks[grp] (callables) are issued after that group's gathers
                are dispatched, so gpsimd-queue collectives land where their
                semaphore waits are already satisfied."""
                if idx_sb is None:
                    idx_sb = load_idx(g, layer)
                Wl = W_sb[layer]
                browl = brow_sb[layer]
                alpha = (al1_sb if layer == 0 else al2_sb)[g]
                if layer == 1:
                    h2all[g] = bigpool.tile([P, nblk * P], bf16, tag="h2all",
                                            name="h2all")
                    st_psum[g] = stpool.tile([1, 2 * P], f32, tag="pst",
                                             name="stpsum")

                nrt = cfg.run_tiles
                for grp in range(cfg.ngrp):
                    nbank = _ceil_banks(grp_sz)
                    banks = [aggpool.tile([P, 4 * P], f32, tag="agg", name="aggbank")
                             for _ in range(nbank)]
                    rv_grp = rpool.tile([1, grp_sz * P], bf16, tag="rv", name="rvgrp")
                    nc.sync.dma_start(
                        rv_grp[:],
                        rinv_t[g][0:1, grp * grp_sz * P:(grp + 1) * grp_sz * P])
                    for b in range(cfg.nbuck):
                        run = grp * cfg.nbuck + b
                        gbuf = gpool.tile([P, nrt, P], bf16, tag="gb", name="gbuf")
                        nc.gpsimd.dma_gather(
                            gbuf[:],
                            src_dram[b * window:(b + 1) * window, :],
                            idx_sb[:, run * (grp_sz * cfg.cell // 16):
                                   (run + 1) * (grp_sz * cfg.cell // 16)],
                            grp_sz * cfg.cell, grp_sz * cfg.cell, P,
                            single_packet=False, queue_num=next_q())
                        # on-chip 0/1 one-hot: oh[lane, t, slot] =
                        #   (sid[lane, t] == slot)
                        ohbuf = ohpool.tile([P, nrt, P], bf16, tag="oh", name="ohbuf")
                        sid_v = sid_sb[g][:, run * nrt:(run + 1) * nrt] \
                            .rearrange("p (t o) -> p t o", o=1) \
                            .broadcast_to([P, nrt, P])
                        iota_v = iota_sb[:].rearrange("p (o f) -> p o f", o=1) \
                            .broadcast_to([P, nrt, P])
                        nc.vector.tensor_tensor(out=ohbuf[:], in0=sid_v,
                                                in1=iota_v, op=OP.is_equal)
                        for c in range(grp_sz):
                            ps = banks[c // 4][:, (c % 4) * P:(c % 4 + 1) * P]
                            for t2 in range(tpc):
                                # start/stop are per 2KB PSUM zero region
                                # (whole bank): only the bank's first matmul
                                # starts, only its last stops.
                                nc.tensor.matmul(
                                    ps, lhsT=gbuf[:, c * tpc + t2, :],
                                    rhs=ohbuf[:, c * tpc + t2, :],
                                    start=(b == 0 and t2 == 0 and c % 4 == 0),
                                    stop=(b == cfg.nbuck - 1 and t2 == tpc - 1
                                          and (c % 4 == 3 or c == grp_sz - 1)))
                    if hooks is not None and grp in hooks:
                        hooks[grp]()
                    # drain the group's blocks
                    aggbf = []
                    for bk in range(nbank):
                        abf = wpool.tile([P, 4 * P], bf16, tag="aggbf", name="aggbf")
                        nc.vector.tensor_copy(abf[:], banks[bk][:])
                        aggbf.append(abf)
                    hgrp = None
                    if layer == 0:
                        hgrp = hgpool.tile([P, grp_sz, P], bf16, tag="hg",
                                           name="hgrp")
                    for c in range(grp_sz):
                        blk = grp * grp_sz + c
                        aggT = aggbf[c // 4][:, (c % 4) * P:(c % 4 + 1) * P]
                        ph = hpool.tile([P, P], f32, tag="ph", name="ph")
                        nc.tensor.matmul(ph[:], lhsT=aggT, rhs=Wl[:],
                                         start=True, stop=False)
                        # rank-1 bias: ph[slot, f] += (1/s_in)[slot] * b[f]
                        nc.tensor.matmul(ph[:],
                                         lhsT=rv_grp[0:1, c * P:(c + 1) * P],
                                         rhs=browl[:], start=False, stop=True)
                        if layer == 0:
                            # h1' = relu(alpha1 * ph), alpha1 = s_out*s_in
                            nc.scalar.activation(hgrp[:, c, :], ph[:], AF.Relu,
                                                 scale=alpha[:, blk:blk + 1])
                        else:
                            nc.scalar.activation(h2all[g][:, blk * P:(blk + 1) * P],
                                                 ph[:], AF.Identity,
                                                 scale=alpha[:, blk:blk + 1])
                    if layer == 0:
                        nc.sync.dma_start(
                            h1loc[g][grp * grp_sz * P:(grp + 1) * grp_sz * P, :]
                            .rearrange("(b p) f -> p b f", p=P),
                            hgrp[:])
                    else:
                        sq = hgpool.tile([P, grp_sz, P], bf16, tag="hg", name="sqgrp")
                        hgs = h2all[g][:, grp * grp_sz * P:(grp + 1) * grp_sz * P]
                        nc.vector.tensor_tensor(
                            out=sq[:].rearrange("p b f -> p (b f)"),
                            in0=hgs, in1=hgs, op=OP.mult)
                        stp = st_psum[g]
                        for c in range(grp_sz):
                            blk = grp * grp_sz + c
                            hslice = h2all[g][:, blk * P:(blk + 1) * P]
                            # both stat slices share one zero region: start
                            # once (sum, blk 0), stop once (sumsq, last blk).
                            nc.tensor.matmul(stp[0:1, 0:P],
                                             lhsT=ones_bf[:, 0:1],
                                             rhs=hslice, start=(blk == 0),
                                             stop=False)
                            nc.tensor.matmul(stp[0:1, P:2 * P],
                                             lhsT=ones_bf[:, 0:1],
                                             rhs=sq[:, c, :], start=False,
                                             stop=(blk == nblk - 1))

            def finish_stats(g):
                """Drain the stats psum and push to HBM (frees the PSUM bank
                and lets the later collective issue without stalling)."""
                st_sb = wpool.tile([1, 2 * P], f32, tag="st", name="stsb")
                nc.vector.tensor_copy(st_sb[:], st_psum[g][:])
                nc.sync.dma_start(stloc[g][:], st_sb[:])

            def finish_collective(g):
                nc.gpsimd.collective_compute(
                    "AllGather", OP.bypass,
                    replica_groups=[list(range(NC))],
                    ins=[stloc[g][:].opt()], outs=[stall[g][:].opt()])

            def finish_graph(g):
                """Global stats + z-score + writeout for graph g."""
                st8 = wpool.tile([NC, 2 * P], f32, tag="st8", name="st8")
                nc.sync.dma_start(st8[:], stall[g][:])
                pred = hpool.tile([1, 2 * P], f32, tag="ph", name="pred")
                nc.tensor.matmul(pred[:], lhsT=ones[0:NC, 0:1], rhs=st8[:],
                                 start=True, stop=True)
                gs = wpool.tile([1, 2 * P], f32, tag="gs", name="gs")
                nc.vector.tensor_copy(gs[:], pred[:])
                ssum = gs[0:1, 0:P]
                ssq = gs[0:1, P:2 * P]
                nv = float(cfg.n)
                # var*(n-1) = ssq - ssum^2/n ; istd = rsqrt(var)
                m2 = wpool.tile([1, P], f32, tag="t0", name="m2")
                nc.vector.tensor_tensor(out=m2[:], in0=ssum, in1=ssum, op=OP.mult)
                nc.vector.tensor_scalar(out=m2[:], in0=m2[:], scalar1=-1.0 / nv,
                                        scalar2=None, op0=OP.mult)
                t1 = wpool.tile([1, P], f32, tag="t1", name="t1")
                nc.vector.tensor_tensor(out=t1[:], in0=ssq, in1=m2[:], op=OP.add)
                std = wpool.tile([1, P], f32, tag="std", name="std")
                nc.scalar.activation(std[:], t1[:], AF.Sqrt,
                                     scale=float(1.0 / (nv - 1.0)))
                istd = wpool.tile([1, P], f32, tag="istd", name="istd")
                nc.vector.reciprocal(istd[:], std[:])
                negms = wpool.tile([1, P], f32, tag="negms", name="negms")
                nc.vector.tensor_tensor(out=negms[:], in0=ssum, in1=istd[:],
                                        op=OP.mult)
                nc.vector.tensor_scalar(out=negms[:], in0=negms[:],
                                        scalar1=-1.0 / nv, scalar2=None,
                                        op0=OP.mult)
                # broadcast istd/negms to [P, P] bf16
                pA = hpool.tile([P, P], f32, tag="ph", name="pA")
                nc.tensor.matmul(pA[:], lhsT=ones[0:1, :], rhs=istd[:],
                                 start=True, stop=True)
                A = wpool.tile([P, P], bf16, tag="A", name="Abc")
                nc.vector.tensor_copy(A[:], pA[:])
                pC = hpool.tile([P, P], f32, tag="ph", name="pC")
                nc.tensor.matmul(pC[:], lhsT=ones[0:1, :], rhs=negms[:],
                                 start=True, stop=True)
                C = wpool.tile([P, P], bf16, tag="C", name="Cbc")
                nc.vector.tensor_copy(C[:], pC[:])
                # z in place over h2all, chunked so DMA-out overlaps the math
                h2 = h2all[g]
                nchunk = 4
                csz = nblk // nchunk
                for ci in range(nchunk):
                    h2v = h2[:, ci * csz * P:(ci + 1) * csz * P] \
                        .rearrange("p (b f) -> p b f", f=P)
                    Abc = A[:].rearrange("p (o f) -> p o f", o=1) \
                        .broadcast_to([P, csz, P])
                    Cbc = C[:].rearrange("p (o f) -> p o f", o=1) \
                        .broadcast_to([P, csz, P])
                    nc.vector.tensor_tensor(out=h2v, in0=h2v, in1=Abc, op=OP.mult)
                    nc.vector.tensor_tensor(out=h2v, in0=h2v, in1=Cbc, op=OP.add)
                    nc.sync.dma_start(
                        z_t[g][ci * csz * P:(ci + 1) * csz * P, :]
                        .rearrange("(b p) f -> p b f", p=P), h2v)

            # --- schedule ---
            # Collectives are issued just after the NEXT pass's first group of
            # gathers: by then their input DMAs have completed, so the shared
            # gpsimd sequencer never stalls the gather stream on a sem wait.
            def ag_h1(g):
                def go():
                    nc.gpsimd.collective_compute(
                        "AllGather", OP.bypass, replica_groups=[list(range(NC))],
                        ins=[h1loc[g][:].opt()], outs=[h1full[g][:].opt()])
                return go

            gather_layer(0, 0, feat_t[0][:], cfg.l1w, idx_sb=idx0_sb)
            gather_layer(1, 0, feat_t[1][:], cfg.l1w, hooks={0: ag_h1(0)})
            gather_layer(0, 1, h1full[0][:], cfg.l2w, hooks={0: ag_h1(1)})
            finish_stats(0)
            gather_layer(1, 1, h1full[1][:], cfg.l2w,
                         hooks={0: lambda: finish_collective(0),
                                3: lambda: finish_graph(0)})
            finish_stats(1)
            finish_collective(1)
            finish_graph(1)

    nc.finalize()
    return nc


# ---------------------------------------------------------------------------
# Entry point
# ---------------------------------------------------------------------------

def _prepare(inputs, cfg_kw=None):
    feat1 = np.asarray(inputs["feat1"], np.float32)
    feat2 = np.asarray(inputs["feat2"], np.float32)
    W1 = np.asarray(inputs["W1"], np.float32)
    b1 = np.asarray(inputs["b1"], np.float32).reshape(1, P)
    W2 = np.asarray(inputs["W2"], np.float32)
    b2 = np.asarray(inputs["b2"], np.float32).reshape(1, P)
    graphs = [(np.asarray(inputs["src1"], np.int64), np.asarray(inputs["dst1"], np.int64)),
              (np.asarray(inputs["src2"], np.int64), np.asarray(inputs["dst2"], np.int64))]

    n = feat1.shape[0]
    base = dict(n=n)
    if cfg_kw:
        base.update(cfg_kw)
    nblk = base.pop("nblk", 104)
    cfg = preps = None
    while True:
        cfg = Cfg(nblk=nblk, **base)
        preps = [_prep_graph(s, d, cfg) for s, d in graphs]
        if all(p is not None for p in preps):
            break
        nblk += cfg.ngrp
        assert nblk * P <= 4 * n, "block packing failed"

    iota = np.tile(np.arange(P, dtype=np.float32).astype(HDT)[None, :], (P, 1))
    # layer-1 gather tables: s_out-prescaled features
    f1b = (feat1 * preps[0]["s_out"][:, None]).astype(HDT)
    f2b = (feat2 * preps[1]["s_out"][:, None]).astype(HDT)
    in_maps = []
    for k in range(NC):
        m = dict(feat1=f1b, feat2=f2b, w1=W1.astype(HDT), w2=W2.astype(HDT),
                 b1=b1, b2=b2, iota=np.ascontiguousarray(iota))
        for g in range(2):
            ck = preps[g]["cores"][k]
            m[f"idxa{g+1}"] = ck["idxA"]
            m[f"idxb{g+1}"] = ck["idxB"]
            m[f"sid{g+1}"] = ck["sid"]
            m[f"al1{g+1}"] = ck["alpha1"]
            m[f"al2{g+1}"] = ck["alpha2"]
            m[f"rinv{g+1}"] = ck["rinv"]
        in_maps.append(m)
    return cfg, preps, in_maps


def _assemble(cfg, preps, results):
    outs = []
    for g in range(2):
        zp = np.concatenate([np.asarray(results[k][f"z{g+1}"])
                             for k in range(NC)], axis=0)
        outs.append(np.ascontiguousarray(zp[preps[g]["rowmap"]]).astype(np.float32))
    return outs[0], outs[1]


def _run(inputs, trace=False, tmpdir=None, cfg_kw=None):
    _ensure_env()
    from concourse import bass_utils

    cfg, preps, in_maps = _prepare(inputs, cfg_kw)
    nc = _build_program(cfg)

    kwargs = {}
    if trace:
        kwargs.update(trace=True)
        if tmpdir:
            kwargs.update(tmpdir=tmpdir)
    res = bass_utils.run_bass_kernel_spmd(nc, in_maps, core_ids=list(range(NC)),
                                          **kwargs)

    z1, z2 = _assemble(cfg, preps, res.results)
    return (z1, z2), res


def kernel(**inputs):
    (z1, z2), _ = _run(inputs)
    return (z1, z2)


# revision 31
# speedup vs baseline: 1.0681x; 1.0681x over previous
"""CCA-SSG GNN (2-layer GraphConv encoder x2 graphs + per-feature z-score) on 8 TRN2 cores.

Strategy (dst-sharded graph parallel):
  - Nodes are partitioned across 8 cores (12500 each, by node-id range).
  - Per core, nodes are packed into NBLK blocks of 128 "slots" such that each
    (block, src-bucket) cell holds <= CELL edges (balanced greedy packing).
  - The SpMM (segment_sum of gathered rows) runs per 128-slot block: edge rows
    are fetched as 128-lane tiles via dma_gather (HBM row gather, 4 SWDGE
    queues) and reduced on the tensor engine against 0/1 one-hot matrices
    BUILT ON-CHIP (DVE is_equal of a resident slot-id table vs an iota row):
    psum[feat, slot] += sum_lane gathered[lane, feat] * OH01[lane, slot].
  - Degree norms are factored out of the one-hot: s_out[src] is pre-folded
    into the gathered tables (host-prescaled feat for layer 1; device-folded
    into h1 rows for layer 2), and s_in[dst] is applied per destination slot
    via the scalar engine's per-partition activation scale. The GraphConv
    bias enters as a rank-1 (K=1) accumulation matmul so a single activation
    computes relu/identity(alpha * (aggW + r*b)).
  - W is applied AFTER aggregation (linearity), so layer-1 gathers prescaled
    bf16 feat rows directly. Layer-2 gathers h1 rows from an AllGather'ed
    bf16 [8*NBLK*128, 128] table (permuted node order).
  - int16 gather indices limit the source window to <32768 rows, so sources
    are split into 4 buckets; each (group-of-blocks, bucket) run is one
    dma_gather.
  - z-score: per-block ones-vector matmuls accumulate sum/sumsq (padding
    slots are exact zeros), tiny AllGather + on-chip reduction, then
    (h2 - mean) * rsqrt(var) per column.
"""

import os
import sys
import types
import numpy as np
import ml_dtypes  # noqa: F401  (registers bfloat16)

P = 128
NC = 8
HDT = np.dtype("float16")

_ENV_READY = False


def _ensure_env():
    global _ENV_READY
    if _ENV_READY:
        return
    if "/root/.axon_site" not in sys.path and os.path.isdir("/root/.axon_site"):
        sys.path.insert(0, "/root/.axon_site")
    # NTFF profiling hook (missing antenv.axon_hooks in container): degrade to
    # no-trace silently if unavailable.
    if "antenv.axon_hooks" not in sys.modules:
        try:
            from trn_agent_boot.trn_boot import _ntff_profile_via_ctypes
            mod = types.ModuleType("antenv.axon_hooks")
            hook = _ntff_profile_via_ctypes("/opt/axon/libaxon_pjrt.so")
            mod.get_axon_ntff_profile_hook = lambda: hook
            mod.set_axon_ntff_profile_hook = lambda h: None
            sys.modules["antenv.axon_hooks"] = mod
        except Exception:
            pass
    _ENV_READY = True


class Cfg:
    def __init__(self, n=100000, ngrp=8, nblk=104, cell=256):
        assert nblk % ngrp == 0
        self.n = n
        self.npc = n // NC               # nodes per core
        self.nbuck = 4
        self.l1w = n // self.nbuck       # layer-1 src bucket window
        self.ngrp = ngrp
        self.nblk = nblk
        self.cell = cell                 # lanes per (block, bucket) cell
        self.grp_sz = nblk // ngrp
        self.nodes_pad = nblk * P
        self.npad = NC * self.nodes_pad
        self.l2w = 2 * self.nodes_pad    # layer-2 bucket window (2 cores)
        self.lanes = nblk * self.nbuck * cell
        self.tiles = self.lanes // P
        self.tpc = cell // P             # tiles per cell
        self.run_tiles = self.grp_sz * self.tpc
        assert self.l1w <= 32767 and self.l2w <= 32767
        assert self.npc <= nblk * P


# ---------------------------------------------------------------------------
# Host-side graph preprocessing
# ---------------------------------------------------------------------------

def _pack_blocks(coords, cfg):
    """Greedy balanced packing of nodes into blocks, vectorized across cores.

    coords: [NC, npc, nbuck] int32 per-(core-local node) bucket in-degree.
    Returns assign [NC, npc] block id, or None if infeasible.
    """
    nblk, cellcap = cfg.nblk, cfg.cell
    tot = coords.sum(2)
    order = np.argsort(-tot, axis=1, kind="stable")
    loads = np.zeros((NC, nblk, cfg.nbuck), np.int32)
    counts = np.zeros((NC, nblk), np.int32)
    assign = np.full((NC, cfg.npc), -1, np.int32)
    ar = np.arange(NC)
    for r in range(cfg.npc):
        nid = order[:, r]
        c = coords[ar, nid]                       # [NC, nbuck]
        cand = loads + c[:, None, :]              # [NC, nblk, nbuck]
        feas = (cand <= cellcap).all(2) & (counts < P)
        if not feas.any(1).all():
            return None
        score = cand.max(2).astype(np.float32) + counts.astype(np.float32) * 1e-3
        score[~feas] = np.inf
        pick = score.argmin(1)
        assign[ar, nid] = pick
        loads[ar, pick] += c
        counts[ar, pick] += 1
    return assign


def _prep_graph(src, dst, cfg):
    """Build per-core device arrays for one graph. Returns dict or None (repack)."""
    n, npc, nblk = cfg.n, cfg.npc, cfg.nblk

    deg_out = np.bincount(src, minlength=n).astype(np.float32)
    deg_in = np.bincount(dst, minlength=n).astype(np.float32)
    s_out = np.maximum(deg_out, 1.0) ** -0.5
    s_in = np.maximum(deg_in, 1.0) ** -0.5

    bkt = src // cfg.l1w                              # [E] edge bucket
    coords_flat = np.bincount(dst * cfg.nbuck + bkt, minlength=n * cfg.nbuck)
    coords = coords_flat.reshape(n, cfg.nbuck).astype(np.int32)
    coords_by_core = coords.reshape(NC, npc, cfg.nbuck)

    assign = _pack_blocks(coords_by_core, cfg)
    if assign is None:
        return None

    # slot within block (stable by local node id), global padded id
    perm_row = np.empty(n, np.int64)                 # node -> block*128 + slot
    blk_counts = np.zeros((NC, nblk), np.int32)
    for k in range(NC):
        a = assign[k]
        sidx = np.argsort(a, kind="stable")
        cnt = np.bincount(a, minlength=nblk)
        starts = np.concatenate([[0], np.cumsum(cnt)[:-1]])
        within = np.arange(npc) - np.repeat(starts, cnt)
        rows = a[sidx] * P + within
        pr = np.empty(npc, np.int64)
        pr[sidx] = rows
        perm_row[k * npc:(k + 1) * npc] = pr
        blk_counts[k] = cnt
    pid = (np.arange(n) // npc) * cfg.nodes_pad + perm_row  # node -> padded row

    cores = {}
    for k in range(NC):
        m = (dst // npc) == k
        es, ed, eb = src[m], dst[m], bkt[m]
        eblk = perm_row[ed] // P
        eslot = perm_row[ed] % P
        # run index in the lane stream: (group, bucket, cell-in-group)
        run = ((eblk // cfg.grp_sz) * cfg.nbuck + eb) * cfg.grp_sz + (eblk % cfg.grp_sz)
        o = np.lexsort((es, run))
        es, eslot, run, eb = es[o], eslot[o], run[o], eb[o]
        cnt = np.bincount(run, minlength=nblk * cfg.nbuck)
        if cnt.max() > cfg.cell:
            return None
        starts = np.concatenate([[0], np.cumsum(cnt)[:-1]])
        within = np.arange(len(run)) - np.repeat(starts, cnt)
        lane = run * cfg.cell + within

        idx1 = np.zeros(cfg.lanes, np.int16)
        idx2 = np.zeros(cfg.lanes, np.int16)
        idx1[lane] = (es - eb * cfg.l1w).astype(np.int16)
        idx2[lane] = (pid[es] - eb.astype(np.int64) * cfg.l2w).astype(np.int16)
        # slot-id table for on-chip one-hot build: [lane%128, tile], -1 = empty
        sid = np.full((P, cfg.tiles), -1.0, HDT)
        sid[lane % P, lane // P] = eslot.astype(HDT)

        idxA = np.tile(idx1.reshape(-1, 16).T, (8, 1)).astype(np.int16)
        idxB = np.tile(idx2.reshape(-1, 16).T, (8, 1)).astype(np.int16)

        # per-slot scale tables [P, nblk]: alpha1 = s_out*s_in (layer-1 out,
        # with next layer's s_out prefold), alpha2 = s_in, rinv = 1/s_in
        # (rank-1 bias coefficient). Zero on padding slots -> exact h = 0.
        nodes = np.arange(k * npc, (k + 1) * npc)
        a1 = np.zeros((P, nblk), np.float32)
        a2 = np.zeros((P, nblk), np.float32)
        rv = np.zeros((P, nblk), np.float32)
        pslot = perm_row[nodes] % P
        pblk = perm_row[nodes] // P
        a1[pslot, pblk] = (s_out[nodes] * s_in[nodes]).astype(np.float32)
        a2[pslot, pblk] = s_in[nodes].astype(np.float32)
        rv[pslot, pblk] = (1.0 / s_in[nodes]).astype(np.float32)
        # rinv used as K=1 matmul lhsT: [1, nblk*P] (slot-major per block)
        rinv = np.ascontiguousarray(rv.T.reshape(1, nblk * P)).astype(HDT)
        cores[k] = dict(idxA=idxA, idxB=idxB, sid=sid,
                        alpha1=a1, alpha2=a2, rinv=rinv)

    rowmap = (np.arange(n) // npc) * cfg.nodes_pad + perm_row
    return dict(cores=cores, rowmap=rowmap, s_out=s_out)


# ---------------------------------------------------------------------------
# Device program
# ---------------------------------------------------------------------------

def _ceil_banks(grp_sz):
    return (grp_sz + 3) // 4


def _build_program(cfg):
    import concourse.bacc as bacc
    import concourse.mybir as mybir
    from concourse.tile import TileContext

    nblk, grp_sz, tpc = cfg.nblk, cfg.grp_sz, cfg.tpc
    f32 = mybir.dt.float32
    bf16 = mybir.dt.float16
    AF = mybir.ActivationFunctionType
    OP = mybir.AluOpType

    NQ = 4
    nc = bacc.Bacc("TRN2", num_devices=NC, target_bir_lowering=False,
                   num_swdge_queues=NQ)

    feat_t = [nc.dram_tensor(f"feat{g+1}", [cfg.n, P], bf16, kind="ExternalInput")
              for g in range(2)]
    W_t = [nc.dram_tensor(f"w{l+1}", [P, P], bf16, kind="ExternalInput")
           for l in range(2)]
    b_t = [nc.dram_tensor(f"b{l+1}", [1, P], f32, kind="ExternalInput")
           for l in range(2)]
    idxA_t = [nc.dram_tensor(f"idxa{g+1}", [P, cfg.lanes // 16], mybir.dt.int16,
                             kind="ExternalInput") for g in range(2)]
    idxB_t = [nc.dram_tensor(f"idxb{g+1}", [P, cfg.lanes // 16], mybir.dt.int16,
                             kind="ExternalInput") for g in range(2)]
    sid_t = [nc.dram_tensor(f"sid{g+1}", [P, cfg.tiles], bf16,
                            kind="ExternalInput") for g in range(2)]
    iota_t = nc.dram_tensor("iota", [P, P], bf16, kind="ExternalInput")
    al1_t = [nc.dram_tensor(f"al1{g+1}", [P, nblk], f32, kind="ExternalInput")
             for g in range(2)]
    al2_t = [nc.dram_tensor(f"al2{g+1}", [P, nblk], f32, kind="ExternalInput")
             for g in range(2)]
    rinv_t = [nc.dram_tensor(f"rinv{g+1}", [1, nblk * P], bf16,
                             kind="ExternalInput") for g in range(2)]
    z_t = [nc.dram_tensor(f"z{g+1}", [cfg.nodes_pad, P], bf16, kind="ExternalOutput")
           for g in range(2)]

    h1loc = [nc.dram_tensor(f"h1loc{g}", [cfg.nodes_pad, P], bf16) for g in range(2)]
    h1full = [nc.dram_tensor(f"h1full{g}", [cfg.npad, P], bf16, addr_space="Shared")
              for g in range(2)]
    stloc = [nc.dram_tensor(f"stloc{g}", [1, 2 * P], f32) for g in range(2)]
    stall = [nc.dram_tensor(f"stall{g}", [NC, 2 * P], f32, addr_space="Shared")
             for g in range(2)]

    qctr = [0]

    def next_q():
        qctr[0] += 1
        return qctr[0] % NQ

    with TileContext(nc) as tc:
        with tc.tile_pool(name="const", bufs=1) as cpool, \
             tc.tile_pool(name="meta", bufs=2) as mpool, \
             tc.tile_pool(name="ohp", bufs=3) as ohpool, \
             tc.tile_pool(name="gb", bufs=8) as gpool, \
             tc.tile_pool(name="rp", bufs=2) as rpool, \
             tc.tile_pool(name="work", bufs=4) as wpool, \
             tc.tile_pool(name="hg", bufs=3) as hgpool, \
             tc.tile_pool(name="big", bufs=2) as bigpool, \
             tc.tile_pool(name="pagg", bufs=_ceil_banks(grp_sz) + 1, space="PSUM") as aggpool, \
             tc.tile_pool(name="ph", bufs=2, space="PSUM") as hpool, \
             tc.tile_pool(name="pst", bufs=1, space="PSUM") as stpool:

            # --- prefetch pass-0 idx before everything else on the DMA queue
            idx0_sb = mpool.tile([P, cfg.lanes // 16], mybir.dt.int16, tag="idx",
                                 name="idxsb")
            nc.sync.dma_start(idx0_sb[:], idxA_t[0][:])

            # --- constants ---
            ones = cpool.tile([P, P], f32, tag="ones")
            nc.vector.memset(ones[:], 1.0)
            ones_bf = cpool.tile([P, P], bf16, tag="onesbf")
            nc.vector.memset(ones_bf[:], 1.0)
            iota_sb = cpool.tile([P, P], bf16, tag="iota")
            nc.sync.dma_start(iota_sb[:], iota_t[:])

            W_sb, brow_sb = [], []
            for l in range(2):
                w = cpool.tile([P, P], bf16, tag=f"w{l}", name=f"wsb{l}")
                nc.sync.dma_start(w[:], W_t[l][:])
                W_sb.append(w)
                brow = cpool.tile([1, P], f32, tag=f"browf{l}", name=f"browf{l}")
                nc.sync.dma_start(brow[:], b_t[l][:])
                brow_bf = cpool.tile([1, P], bf16, tag=f"brow{l}", name=f"brow{l}")
                nc.vector.tensor_copy(brow_bf[:], brow[:])
                brow_sb.append(brow_bf)

            sid_sb, al1_sb, al2_sb = [], [], []
            for g in range(2):
                s = cpool.tile([P, cfg.tiles], bf16, tag=f"sid{g}", name=f"sid{g}")
                nc.sync.dma_start(s[:], sid_t[g][:])
                sid_sb.append(s)
                a1 = cpool.tile([P, nblk], f32, tag=f"al1{g}", name=f"al1{g}")
                nc.sync.dma_start(a1[:], al1_t[g][:])
                al1_sb.append(a1)
                a2 = cpool.tile([P, nblk], f32, tag=f"al2{g}", name=f"al2{g}")
                nc.sync.dma_start(a2[:], al2_t[g][:])
                al2_sb.append(a2)

            h2all = [None, None]
            st_psum = [None, None]

            def load_idx(g, layer):
                idx_dram = (idxA_t if layer == 0 else idxB_t)[g]
                idx_sb = mpool.tile([P, cfg.lanes // 16], mybir.dt.int16, tag="idx",
                                    name="idxsb")
                nc.sync.dma_start(idx_sb[:], idx_dram[:])
                return idx_sb

            def gather_layer(g, layer, src_dram, window, idx_sb=None,
                             hooks=None):
                """One gather+aggregate layer for graph g. Produces per-block
                h into h1loc dram (layer 0) or h2all sbuf + stats (layer 1).
                hooks[grp] (callables) are issued after that group's gathers
                are dispatched, so gpsimd-queue collectives land where their
                semaphore waits are already satisfied."""
                if idx_sb is None:
                    idx_sb = load_idx(g, layer)
                Wl = W_sb[layer]
                browl = brow_sb[layer]
                alpha = (al1_sb if layer == 0 else al2_sb)[g]
                if layer == 1:
                    h2all[g] = bigpool.tile([P, nblk * P], bf16, tag="h2all",
                                            name="h2all")
                    st_psum[g] = stpool.tile([1, 2 * P], f32, tag="pst",
                                             name="stpsum")

                nrt = cfg.run_tiles
                for grp in range(cfg.ngrp):
                    nbank = _ceil_banks(grp_sz)
                    banks = [aggpool.tile([P, 4 * P], f32, tag="agg", name="aggbank")
                             for _ in range(nbank)]
                    rv_grp = rpool.tile([1, grp_sz * P], bf16, tag="rv", name="rvgrp")
                    nc.sync.dma_start(
                        rv_grp[:],
                        rinv_t[g][0:1, grp * grp_sz * P:(grp + 1) * grp_sz * P])
                    for b in range(cfg.nbuck):
                        run = grp * cfg.nbuck + b
                        gbuf = gpool.tile([P, nrt, P], bf16, tag="gb", name="gbuf")
                        nc.gpsimd.dma_gather(
                            gbuf[:],
                            src_dram[b * window:(b + 1) * window, :],
                            idx_sb[:, run * (grp_sz * cfg.cell // 16):
                                   (run + 1) * (grp_sz * cfg.cell // 16)],
                            grp_sz * cfg.cell, grp_sz * cfg.cell, P,
                            single_packet=False, queue_num=next_q())
                        # on-chip 0/1 one-hot: oh[lane, t, slot] =
                        #   (sid[lane, t] == slot)
                        ohbuf = ohpool.tile([P, nrt, P], bf16, tag="oh", name="ohbuf")
                        sid_v = sid_sb[g][:, run * nrt:(run + 1) * nrt] \
                            .rearrange("p (t o) -> p t o", o=1) \
                            .broadcast_to([P, nrt, P])
                        iota_v = iota_sb[:].rearrange("p (o f) -> p o f", o=1) \
                            .broadcast_to([P, nrt, P])
                        nc.vector.tensor_tensor(out=ohbuf[:], in0=sid_v,
                                                in1=iota_v, op=OP.is_equal)
                        for c in range(grp_sz):
                            ps = banks[c // 4][:, (c % 4) * P:(c % 4 + 1) * P]
                            for t2 in range(tpc):
                                # start/stop are per 2KB PSUM zero region
                                # (whole bank): only the bank's first matmul
                                # starts, only its last stops.
                                nc.tensor.matmul(
                                    ps, lhsT=gbuf[:, c * tpc + t2, :],
                                    rhs=ohbuf[:, c * tpc + t2, :],
                                    start=(b == 0 and t2 == 0 and c % 4 == 0),
                                    stop=(b == cfg.nbuck - 1 and t2 == tpc - 1
                                          and (c % 4 == 3 or c == grp_sz - 1)))
                    if hooks is not None and grp in hooks:
                        hooks[grp]()
                    # drain the group's blocks
                    aggbf = []
                    for bk in range(nbank):
                        abf = wpool.tile([P, 4 * P], bf16, tag="aggbf", name="aggbf")
                        nc.vector.tensor_copy(abf[:], banks[bk][:])
                        aggbf.append(abf)
                    hgrp = None
                    if layer == 0:
                        hgrp = hgpool.tile([P, grp_sz, P], bf16, tag="hg",
                                           name="hgrp")
                    for c in range(grp_sz):
                        blk = grp * grp_sz + c
                        aggT = aggbf[c // 4][:, (c % 4) * P:(c % 4 + 1) * P]
                        ph = hpool.tile([P, P], f32, tag="ph", name="ph")
                        nc.tensor.matmul(ph[:], lhsT=aggT, rhs=Wl[:],
                                         start=True, stop=False)
                        # rank-1 bias: ph[slot, f] += (1/s_in)[slot] * b[f]
                        nc.tensor.matmul(ph[:],
                                         lhsT=rv_grp[0:1, c * P:(c + 1) * P],
                                         rhs=browl[:], start=False, stop=True)
                        if layer == 0:
                            # h1' = relu(alpha1 * ph), alpha1 = s_out*s_in
                            nc.scalar.activation(hgrp[:, c, :], ph[:], AF.Relu,
                                                 scale=alpha[:, blk:blk + 1])
                        else:
                            nc.scalar.activation(h2all[g][:, blk * P:(blk + 1) * P],
                                                 ph[:], AF.Identity,
                                                 scale=alpha[:, blk:blk + 1])
                    if layer == 0:
                        nc.sync.dma_start(
                            h1loc[g][grp * grp_sz * P:(grp + 1) * grp_sz * P, :]
                            .rearrange("(b p) f -> p b f", p=P),
                            hgrp[:])
                    else:
                        sq = hgpool.tile([P, grp_sz, P], bf16, tag="hg", name="sqgrp")
                        hgs = h2all[g][:, grp * grp_sz * P:(grp + 1) * grp_sz * P]
                        nc.vector.tensor_tensor(
                            out=sq[:].rearrange("p b f -> p (b f)"),
                            in0=hgs, in1=hgs, op=OP.mult)
                        stp = st_psum[g]
                        for c in range(grp_sz):
                            blk = grp * grp_sz + c
                            hslice = h2all[g][:, blk * P:(blk + 1) * P]
                            # both stat slices share one zero region: start
                            # once (sum, blk 0), stop once (sumsq, last blk).
                            nc.tensor.matmul(stp[0:1, 0:P],
                                             lhsT=ones_bf[:, 0:1],
                                             rhs=hslice, start=(blk == 0),
                                             stop=False)
                            nc.tensor.matmul(stp[0:1, P:2 * P],
                                             lhsT=ones_bf[:, 0:1],
                                             rhs=sq[:, c, :], start=False,
                                             stop=(blk == nblk - 1))

            def finish_stats(g):
                """Drain the stats psum and push to HBM (frees the PSUM bank
                and lets the later collective issue without stalling)."""
                st_sb = wpool.tile([1, 2 * P], f32, tag="st", name="stsb")
                nc.vector.tensor_copy(st_sb[:], st_psum[g][:])
                nc.sync.dma_start(stloc[g][:], st_sb[:])

            def finish_collective(g):
                nc.gpsimd.collective_compute(
                    "AllGather", OP.bypass,
                    replica_groups=[list(range(NC))],
                    ins=[stloc[g][:].opt()], outs=[stall[g][:].opt()])

            def finish_graph(g):
                """Global stats + z-score + writeout for graph g."""
                st8 = wpool.tile([NC, 2 * P], f32, tag="st8", name="st8")
                nc.sync.dma_start(st8[:], stall[g][:])
                pred = hpool.tile([1, 2 * P], f32, tag="ph", name="pred")
                nc.tensor.matmul(pred[:], lhsT=ones[0:NC, 0:1], rhs=st8[:],
                                 start=True, stop=True)
                gs = wpool.tile([1, 2 * P], f32, tag="gs", name="gs")
                nc.vector.tensor_copy(gs[:], pred[:])
                ssum = gs[0:1, 0:P]
                ssq = gs[0:1, P:2 * P]
                nv = float(cfg.n)
                # var*(n-1) = ssq - ssum^2/n ; istd = rsqrt(var)
                m2 = wpool.tile([1, P], f32, tag="t0", name="m2")
                nc.vector.tensor_tensor(out=m2[:], in0=ssum, in1=ssum, op=OP.mult)
                nc.vector.tensor_scalar(out=m2[:], in0=m2[:], scalar1=-1.0 / nv,
                                        scalar2=None, op0=OP.mult)
                t1 = wpool.tile([1, P], f32, tag="t1", name="t1")
                nc.vector.tensor_tensor(out=t1[:], in0=ssq, in1=m2[:], op=OP.add)
                std = wpool.tile([1, P], f32, tag="std", name="std")
                nc.scalar.activation(std[:], t1[:], AF.Sqrt,
                                     scale=float(1.0 / (nv - 1.0)))
                istd = wpool.tile([1, P], f32, tag="istd", name="istd")
                nc.vector.reciprocal(istd[:], std[:])
                negms = wpool.tile([1, P], f32, tag="negms", name="negms")
                nc.vector.tensor_tensor(out=negms[:], in0=ssum, in1=istd[:],
                                        op=OP.mult)
                nc.vector.tensor_scalar(out=negms[:], in0=negms[:],
                                        scalar1=-1.0 / nv, scalar2=None,
                                        op0=OP.mult)
                # broadcast istd/negms to [P, P] bf16
                pA = hpool.tile([P, P], f32, tag="ph", name="pA")
                nc.tensor.matmul(pA[:], lhsT=ones[0:1, :], rhs=istd[:],
                                 start=True, stop=True)
                A = wpool.tile([P, P], bf16, tag="A", name="Abc")
                nc.vector.tensor_copy(A[:], pA[:])
                pC = hpool.tile([P, P], f32, tag="ph", name="pC")
                nc.tensor.matmul(pC[:], lhsT=ones[0:1, :], rhs=negms[:],
                                 start=True, stop=True)
                C = wpool.tile([P, P], bf16, tag="C", name="Cbc")
                nc.vector.tensor_copy(C[:], pC[:])
                # z in place over h2all, chunked so DMA-out overlaps the math
                h2 = h2all[g]
                nchunk = 4
                csz = nblk // nchunk
                for ci in range(nchunk):
                    h2v = h2[:, ci * csz * P:(ci + 1) * csz * P] \
                        .rearrange("p (b f) -> p b f", f=P)
                    Abc = A[:].rearrange("p (o f) -> p o f", o=1) \
                        .broadcast_to([P, csz, P])
                    Cbc = C[:].rearrange("p (o f) -> p o f", o=1) \
                        .broadcast_to([P, csz, P])
                    nc.vector.tensor_tensor(out=h2v, in0=h2v, in1=Abc, op=OP.mult)
                    nc.vector.tensor_tensor(out=h2v, in0=h2v, in1=Cbc, op=OP.add)
                    nc.sync.dma_start(
                        z_t[g][ci * csz * P:(ci + 1) * csz * P, :]
                        .rearrange("(b p) f -> p b f", p=P), h2v)

            # --- schedule ---
            # Collectives are issued just after the NEXT pass's first group of
            # gathers: by then their input DMAs have completed, so the shared
            # gpsimd sequencer never stalls the gather stream on a sem wait.
            def ag_h1(g):
                def go():
                    nc.gpsimd.collective_compute(
                        "AllGather", OP.bypass, replica_groups=[list(range(NC))],
                        ins=[h1loc[g][:].opt()], outs=[h1full[g][:].opt()])
                return go

            gather_layer(0, 0, feat_t[0][:], cfg.l1w, idx_sb=idx0_sb)
            gather_layer(1, 0, feat_t[1][:], cfg.l1w, hooks={0: ag_h1(0)})
            gather_layer(0, 1, h1full[0][:], cfg.l2w, hooks={0: ag_h1(1)})
            finish_stats(0)
            gather_layer(1, 1, h1full[1][:], cfg.l2w,
                         hooks={0: lambda: finish_collective(0),
                                3: lambda: finish_graph(0)})
            finish_stats(1)
            finish_collective(1)
            finish_graph(1)

    nc.finalize()
    return nc


# ---------------------------------------------------------------------------
# Entry point
# ---------------------------------------------------------------------------

def _prepare(inputs, cfg_kw=None):
    feat1 = np.asarray(inputs["feat1"], np.float32)
    feat2 = np.asarray(inputs["feat2"], np.float32)
    W1 = np.asarray(inputs["W1"], np.float32)
    b1 = np.asarray(inputs["b1"], np.float32).reshape(1, P)
    W2 = np.asarray(inputs["W2"], np.float32)
    b2 = np.asarray(inputs["b2"], np.float32).reshape(1, P)
    graphs = [(np.asarray(inputs["src1"], np.int64), np.asarray(inputs["dst1"], np.int64)),
              (np.asarray(inputs["src2"], np.int64), np.asarray(inputs["dst2"], np.int64))]

    n = feat1.shape[0]
    base = dict(n=n)
    if cfg_kw:
        base.update(cfg_kw)
    nblk = base.pop("nblk", 104)
    cfg = preps = None
    while True:
        cfg = Cfg(nblk=nblk, **base)
        preps = [_prep_graph(s, d, cfg) for s, d in graphs]
        if all(p is not None for p in preps):
            break
        nblk += cfg.ngrp
        assert nblk * P <= 4 * n, "block packing failed"

    iota = np.tile(np.arange(P, dtype=np.float32).astype(HDT)[None, :], (P, 1))
    # layer-1 gather tables: s_out-prescaled features
    f1b = (feat1 * preps[0]["s_out"][:, None]).astype(HDT)
    f2b = (feat2 * preps[1]["s_out"][:, None]).astype(HDT)
    in_maps = []
    for k in range(NC):
        m = dict(feat1=f1b, feat2=f2b, w1=W1.astype(HDT), w2=W2.astype(HDT),
                 b1=b1, b2=b2, iota=np.ascontiguousarray(iota))
        for g in range(2):
            ck = preps[g]["cores"][k]
            m[f"idxa{g+1}"] = ck["idxA"]
            m[f"idxb{g+1}"] = ck["idxB"]
            m[f"sid{g+1}"] = ck["sid"]
            m[f"al1{g+1}"] = ck["alpha1"]
            m[f"al2{g+1}"] = ck["alpha2"]
            m[f"rinv{g+1}"] = ck["rinv"]
        in_maps.append(m)
    return cfg, preps, in_maps


def _assemble(cfg, preps, results):
    outs = []
    for g in range(2):
        zp = np.concatenate([np.asarray(results[k][f"z{g+1}"])
                             for k in range(NC)], axis=0)
        outs.append(np.ascontiguousarray(zp[preps[g]["rowmap"]]).astype(np.float32))
    return outs[0], outs[1]


def _run(inputs, trace=False, tmpdir=None, cfg_kw=None):
    _ensure_env()
    from concourse import bass_utils

    cfg, preps, in_maps = _prepare(inputs, cfg_kw)
    nc = _build_program(cfg)

    kwargs = {}
    if trace:
        kwargs.update(trace=True)
        if tmpdir:
            kwargs.update(tmpdir=tmpdir)
    res = bass_utils.run_bass_kernel_spmd(nc, in_maps, core_ids=list(range(NC)),
                                          **kwargs)

    z1, z2 = _assemble(cfg, preps, res.results)
    return (z1, z2), res


def kernel(**inputs):
    (z1, z2), _ = _run(inputs)
    return (z1, z2)


# revision 33
# speedup vs baseline: 1.1293x; 1.0573x over previous
"""CCA-SSG GNN (2-layer GraphConv encoder x2 graphs + per-feature z-score) on 8 TRN2 cores.

Strategy (dst-sharded graph parallel):
  - Nodes are partitioned across 8 cores (12500 each, by node-id range).
  - Per core, nodes are packed into NBLK blocks of 128 "slots" such that each
    (block, src-bucket) cell holds <= CELL edges (balanced greedy packing).
  - The SpMM (segment_sum of gathered rows) runs per 128-slot block: edge rows
    are fetched as 128-lane tiles via dma_gather (HBM row gather, 4 SWDGE
    queues) and reduced on the tensor engine against 0/1 one-hot matrices
    BUILT ON-CHIP (DVE is_equal of a resident slot-id table vs an iota row):
    psum[feat, slot] += sum_lane gathered[lane, feat] * OH01[lane, slot].
  - Degree norms are factored out of the one-hot: s_out[src] is pre-folded
    into the gathered tables (host-prescaled feat for layer 1; device-folded
    into h1 rows for layer 2), and s_in[dst] is applied per destination slot
    via the scalar engine's per-partition activation scale. The GraphConv
    bias enters as a rank-1 (K=1) accumulation matmul so a single activation
    computes relu/identity(alpha * (aggW + r*b)).
  - W is applied AFTER aggregation (linearity), so layer-1 gathers prescaled
    bf16 feat rows directly. Layer-2 gathers h1 rows from an AllGather'ed
    bf16 [8*NBLK*128, 128] table (permuted node order).
  - int16 gather indices limit the source window to <32768 rows, so sources
    are split into 4 buckets; each (group-of-blocks, bucket) run is one
    dma_gather.
  - z-score: per-block ones-vector matmuls accumulate sum/sumsq (padding
    slots are exact zeros), tiny AllGather + on-chip reduction, then
    (h2 - mean) * rsqrt(var) per column.
"""

import os
import sys
import types
import numpy as np
import ml_dtypes  # noqa: F401  (registers bfloat16)

P = 128
NC = 8
HDT = np.dtype("float16")

_ENV_READY = False


def _ensure_env():
    global _ENV_READY
    if _ENV_READY:
        return
    if "/root/.axon_site" not in sys.path and os.path.isdir("/root/.axon_site"):
        sys.path.insert(0, "/root/.axon_site")
    # NTFF profiling hook (missing antenv.axon_hooks in container): degrade to
    # no-trace silently if unavailable.
    if "antenv.axon_hooks" not in sys.modules:
        try:
            from trn_agent_boot.trn_boot import _ntff_profile_via_ctypes
            mod = types.ModuleType("antenv.axon_hooks")
            hook = _ntff_profile_via_ctypes("/opt/axon/libaxon_pjrt.so")
            mod.get_axon_ntff_profile_hook = lambda: hook
            mod.set_axon_ntff_profile_hook = lambda h: None
            sys.modules["antenv.axon_hooks"] = mod
        except Exception:
            pass
    _ENV_READY = True


class Cfg:
    def __init__(self, n=100000, ngrp=8, nblk=104, cell=256):
        assert nblk % ngrp == 0
        self.n = n
        self.npc = n // NC               # nodes per core
        self.nbuck = 4
        self.l1w = n // self.nbuck       # layer-1 src bucket window
        self.ngrp = ngrp
        self.nblk = nblk
        self.cell = cell                 # lanes per (block, bucket) cell
        self.grp_sz = nblk // ngrp
        self.nodes_pad = nblk * P
        self.npad = NC * self.nodes_pad
        self.l2w = 2 * self.nodes_pad    # layer-2 bucket window (2 cores)
        self.lanes = nblk * self.nbuck * cell
        self.tiles = self.lanes // P
        self.tpc = cell // P             # tiles per cell
        self.run_tiles = self.grp_sz * self.tpc
        assert self.l1w <= 32767 and self.l2w <= 32767
        assert self.npc <= nblk * P


# ---------------------------------------------------------------------------
# Host-side graph preprocessing
# ---------------------------------------------------------------------------

def _pack_blocks(coords, cfg):
    """Greedy balanced packing of nodes into blocks, vectorized across cores.

    coords: [NC, npc, nbuck] int32 per-(core-local node) bucket in-degree.
    Returns assign [NC, npc] block id, or None if infeasible.
    """
    nblk, cellcap = cfg.nblk, cfg.cell
    tot = coords.sum(2)
    order = np.argsort(-tot, axis=1, kind="stable")
    loads = np.zeros((NC, nblk, cfg.nbuck), np.int32)
    counts = np.zeros((NC, nblk), np.int32)
    assign = np.full((NC, cfg.npc), -1, np.int32)
    ar = np.arange(NC)
    for r in range(cfg.npc):
        nid = order[:, r]
        c = coords[ar, nid]                       # [NC, nbuck]
        cand = loads + c[:, None, :]              # [NC, nblk, nbuck]
        feas = (cand <= cellcap).all(2) & (counts < P)
        if not feas.any(1).all():
            return None
        score = cand.max(2).astype(np.float32) + counts.astype(np.float32) * 1e-3
        score[~feas] = np.inf
        pick = score.argmin(1)
        assign[ar, nid] = pick
        loads[ar, pick] += c
        counts[ar, pick] += 1
    return assign


def _prep_graph(src, dst, cfg):
    """Build per-core device arrays for one graph. Returns dict or None (repack)."""
    n, npc, nblk = cfg.n, cfg.npc, cfg.nblk

    deg_out = np.bincount(src, minlength=n).astype(np.float32)
    deg_in = np.bincount(dst, minlength=n).astype(np.float32)
    s_out = np.maximum(deg_out, 1.0) ** -0.5
    s_in = np.maximum(deg_in, 1.0) ** -0.5

    bkt = src // cfg.l1w                              # [E] edge bucket
    coords_flat = np.bincount(dst * cfg.nbuck + bkt, minlength=n * cfg.nbuck)
    coords = coords_flat.reshape(n, cfg.nbuck).astype(np.int32)
    coords_by_core = coords.reshape(NC, npc, cfg.nbuck)

    assign = _pack_blocks(coords_by_core, cfg)
    if assign is None:
        return None

    # slot within block (stable by local node id), global padded id
    perm_row = np.empty(n, np.int64)                 # node -> block*128 + slot
    blk_counts = np.zeros((NC, nblk), np.int32)
    for k in range(NC):
        a = assign[k]
        sidx = np.argsort(a, kind="stable")
        cnt = np.bincount(a, minlength=nblk)
        starts = np.concatenate([[0], np.cumsum(cnt)[:-1]])
        within = np.arange(npc) - np.repeat(starts, cnt)
        rows = a[sidx] * P + within
        pr = np.empty(npc, np.int64)
        pr[sidx] = rows
        perm_row[k * npc:(k + 1) * npc] = pr
        blk_counts[k] = cnt
    pid = (np.arange(n) // npc) * cfg.nodes_pad + perm_row  # node -> padded row

    cores = {}
    for k in range(NC):
        m = (dst // npc) == k
        es, ed, eb = src[m], dst[m], bkt[m]
        eblk = perm_row[ed] // P
        eslot = perm_row[ed] % P
        # run index in the lane stream: (group, bucket, cell-in-group)
        run = ((eblk // cfg.grp_sz) * cfg.nbuck + eb) * cfg.grp_sz + (eblk % cfg.grp_sz)
        o = np.lexsort((es, run))
        es, eslot, run, eb = es[o], eslot[o], run[o], eb[o]
        cnt = np.bincount(run, minlength=nblk * cfg.nbuck)
        if cnt.max() > cfg.cell:
            return None
        starts = np.concatenate([[0], np.cumsum(cnt)[:-1]])
        within = np.arange(len(run)) - np.repeat(starts, cnt)
        lane = run * cfg.cell + within

        idx1 = np.zeros(cfg.lanes, np.int16)
        idx2 = np.zeros(cfg.lanes, np.int16)
        idx1[lane] = (es - eb * cfg.l1w).astype(np.int16)
        idx2[lane] = (pid[es] - eb.astype(np.int64) * cfg.l2w).astype(np.int16)
        # slot-id table for on-chip one-hot build: [lane%128, tile], -1 = empty
        sid = np.full((P, cfg.tiles), -1.0, HDT)
        sid[lane % P, lane // P] = eslot.astype(HDT)

        idxA = np.tile(idx1.reshape(-1, 16).T, (8, 1)).astype(np.int16)
        idxB = np.tile(idx2.reshape(-1, 16).T, (8, 1)).astype(np.int16)

        # per-slot scale tables [P, nblk]: alpha1 = s_out*s_in (layer-1 out,
        # with next layer's s_out prefold), alpha2 = s_in, rinv = 1/s_in
        # (rank-1 bias coefficient). Zero on padding slots -> exact h = 0.
        nodes = np.arange(k * npc, (k + 1) * npc)
        a1 = np.zeros((P, nblk), np.float32)
        a2 = np.zeros((P, nblk), np.float32)
        rv = np.zeros((P, nblk), np.float32)
        pslot = perm_row[nodes] % P
        pblk = perm_row[nodes] // P
        a1[pslot, pblk] = (s_out[nodes] * s_in[nodes]).astype(np.float32)
        a2[pslot, pblk] = s_in[nodes].astype(np.float32)
        rv[pslot, pblk] = (1.0 / s_in[nodes]).astype(np.float32)
        # rinv used as K=1 matmul lhsT: [1, nblk*P] (slot-major per block)
        rinv = np.ascontiguousarray(rv.T.reshape(1, nblk * P)).astype(HDT)
        cores[k] = dict(idxA=idxA, idxB=idxB, sid=sid,
                        alpha1=a1, alpha2=a2, rinv=rinv)

    rowmap = (np.arange(n) // npc) * cfg.nodes_pad + perm_row
    return dict(cores=cores, rowmap=rowmap, s_out=s_out)


# ---------------------------------------------------------------------------
# Device program
# ---------------------------------------------------------------------------

def _ceil_banks(grp_sz):
    return (grp_sz + 3) // 4


def _build_program(cfg):
    import concourse.bacc as bacc
    import concourse.mybir as mybir
    from concourse.tile import TileContext

    nblk, grp_sz, tpc = cfg.nblk, cfg.grp_sz, cfg.tpc
    f32 = mybir.dt.float32
    bf16 = mybir.dt.float16
    AF = mybir.ActivationFunctionType
    OP = mybir.AluOpType

    NQ = 4
    nc = bacc.Bacc("TRN2", num_devices=NC, target_bir_lowering=False,
                   num_swdge_queues=NQ)

    feat_t = [nc.dram_tensor(f"feat{g+1}", [cfg.n, P], bf16, kind="ExternalInput")
              for g in range(2)]
    W_t = [nc.dram_tensor(f"w{l+1}", [P, P], bf16, kind="ExternalInput")
           for l in range(2)]
    b_t = [nc.dram_tensor(f"b{l+1}", [1, P], f32, kind="ExternalInput")
           for l in range(2)]
    idxA_t = [nc.dram_tensor(f"idxa{g+1}", [P, cfg.lanes // 16], mybir.dt.int16,
                             kind="ExternalInput") for g in range(2)]
    idxB_t = [nc.dram_tensor(f"idxb{g+1}", [P, cfg.lanes // 16], mybir.dt.int16,
                             kind="ExternalInput") for g in range(2)]
    sid_t = [nc.dram_tensor(f"sid{g+1}", [P, cfg.tiles], bf16,
                            kind="ExternalInput") for g in range(2)]
    iota_t = nc.dram_tensor("iota", [P, P], bf16, kind="ExternalInput")
    al1_t = [nc.dram_tensor(f"al1{g+1}", [P, nblk], f32, kind="ExternalInput")
             for g in range(2)]
    al2_t = [nc.dram_tensor(f"al2{g+1}", [P, nblk], f32, kind="ExternalInput")
             for g in range(2)]
    rinv_t = [nc.dram_tensor(f"rinv{g+1}", [1, nblk * P], bf16,
                             kind="ExternalInput") for g in range(2)]
    z_t = [nc.dram_tensor(f"z{g+1}", [cfg.nodes_pad, P], bf16, kind="ExternalOutput")
           for g in range(2)]

    h1loc = [nc.dram_tensor(f"h1loc{g}", [cfg.nodes_pad, P], bf16) for g in range(2)]
    h1full = [nc.dram_tensor(f"h1full{g}", [cfg.npad, P], bf16, addr_space="Shared")
              for g in range(2)]
    stloc = [nc.dram_tensor(f"stloc{g}", [1, 2 * P], f32) for g in range(2)]
    stall = [nc.dram_tensor(f"stall{g}", [NC, 2 * P], f32, addr_space="Shared")
             for g in range(2)]

    qctr = [0]

    def next_q():
        qctr[0] += 1
        return qctr[0] % NQ

    with TileContext(nc) as tc:
        with tc.tile_pool(name="const", bufs=1) as cpool, \
             tc.tile_pool(name="meta", bufs=2) as mpool, \
             tc.tile_pool(name="ohp", bufs=3) as ohpool, \
             tc.tile_pool(name="gb", bufs=8) as gpool, \
             tc.tile_pool(name="rp", bufs=2) as rpool, \
             tc.tile_pool(name="work", bufs=4) as wpool, \
             tc.tile_pool(name="hg", bufs=3) as hgpool, \
             tc.tile_pool(name="sq", bufs=3) as sqpool, \
             tc.tile_pool(name="big", bufs=2) as bigpool, \
             tc.tile_pool(name="pagg", bufs=_ceil_banks(grp_sz) + 1, space="PSUM") as aggpool, \
             tc.tile_pool(name="ph", bufs=2, space="PSUM") as hpool, \
             tc.tile_pool(name="pst", bufs=1, space="PSUM") as stpool:

            # --- prefetch pass-0 idx before everything else on the DMA queue
            idx0_sb = mpool.tile([P, cfg.lanes // 16], mybir.dt.int16, tag="idx",
                                 name="idxsb")
            nc.sync.dma_start(idx0_sb[:], idxA_t[0][:])

            # --- constants ---
            ones = cpool.tile([P, P], f32, tag="ones")
            nc.vector.memset(ones[:], 1.0)
            ones_bf = cpool.tile([P, P], bf16, tag="onesbf")
            nc.vector.memset(ones_bf[:], 1.0)
            iota_sb = cpool.tile([P, P], bf16, tag="iota")
            nc.sync.dma_start(iota_sb[:], iota_t[:])

            W_sb, brow_sb = [], []
            for l in range(2):
                w = cpool.tile([P, P], bf16, tag=f"w{l}", name=f"wsb{l}")
                nc.sync.dma_start(w[:], W_t[l][:])
                W_sb.append(w)
                brow = cpool.tile([1, P], f32, tag=f"browf{l}", name=f"browf{l}")
                nc.sync.dma_start(brow[:], b_t[l][:])
                brow_bf = cpool.tile([1, P], bf16, tag=f"brow{l}", name=f"brow{l}")
                nc.vector.tensor_copy(brow_bf[:], brow[:])
                brow_sb.append(brow_bf)

            sid_sb, al1_sb, al2_sb = [], [], []
            for g in range(2):
                s = cpool.tile([P, cfg.tiles], bf16, tag=f"sid{g}", name=f"sid{g}")
                nc.sync.dma_start(s[:], sid_t[g][:])
                sid_sb.append(s)
                a1 = cpool.tile([P, nblk], f32, tag=f"al1{g}", name=f"al1{g}")
                nc.sync.dma_start(a1[:], al1_t[g][:])
                al1_sb.append(a1)
                a2 = cpool.tile([P, nblk], f32, tag=f"al2{g}", name=f"al2{g}")
                nc.sync.dma_start(a2[:], al2_t[g][:])
                al2_sb.append(a2)

            h2all = [None, None]
            st_psum = [None, None]

            def load_idx(g, layer):
                idx_dram = (idxA_t if layer == 0 else idxB_t)[g]
                idx_sb = mpool.tile([P, cfg.lanes // 16], mybir.dt.int16, tag="idx",
                                    name="idxsb")
                nc.sync.dma_start(idx_sb[:], idx_dram[:])
                return idx_sb

            def gather_layer(g, layer, src_dram, window, idx_sb=None,
                             hooks=None):
                """One gather+aggregate layer for graph g. Produces per-block
                h into h1loc dram (layer 0) or h2all sbuf + stats (layer 1).
                hooks[grp] (callables) are issued after that group's gathers
                are dispatched, so gpsimd-queue collectives land where their
                semaphore waits are already satisfied."""
                if idx_sb is None:
                    idx_sb = load_idx(g, layer)
                Wl = W_sb[layer]
                browl = brow_sb[layer]
                alpha = (al1_sb if layer == 0 else al2_sb)[g]
                if layer == 1:
                    h2all[g] = bigpool.tile([P, nblk * P], bf16, tag="h2all",
                                            name="h2all")
                    st_psum[g] = stpool.tile([1, 2 * P], f32, tag="pst",
                                             name="stpsum")

                nrt = cfg.run_tiles
                for grp in range(cfg.ngrp):
                    nbank = _ceil_banks(grp_sz)
                    banks = [aggpool.tile([P, 4 * P], f32, tag="agg", name="aggbank")
                             for _ in range(nbank)]
                    rv_grp = rpool.tile([1, grp_sz * P], bf16, tag="rv", name="rvgrp")
                    nc.sync.dma_start(
                        rv_grp[:],
                        rinv_t[g][0:1, grp * grp_sz * P:(grp + 1) * grp_sz * P])
                    for b in range(cfg.nbuck):
                        run = grp * cfg.nbuck + b
                        gbuf = gpool.tile([P, nrt, P], bf16, tag="gb", name="gbuf")
                        nc.gpsimd.dma_gather(
                            gbuf[:],
                            src_dram[b * window:(b + 1) * window, :],
                            idx_sb[:, run * (grp_sz * cfg.cell // 16):
                                   (run + 1) * (grp_sz * cfg.cell // 16)],
                            grp_sz * cfg.cell, grp_sz * cfg.cell, P,
                            single_packet=False, queue_num=next_q())
                        # on-chip 0/1 one-hot: oh[lane, t, slot] =
                        #   (sid[lane, t] == slot)
                        ohbuf = ohpool.tile([P, nrt, P], bf16, tag="oh", name="ohbuf")
                        sid_v = sid_sb[g][:, run * nrt:(run + 1) * nrt] \
                            .rearrange("p (t o) -> p t o", o=1) \
                            .broadcast_to([P, nrt, P])
                        iota_v = iota_sb[:].rearrange("p (o f) -> p o f", o=1) \
                            .broadcast_to([P, nrt, P])
                        nc.vector.tensor_tensor(out=ohbuf[:], in0=sid_v,
                                                in1=iota_v, op=OP.is_equal)
                        for c in range(grp_sz):
                            ps = banks[c // 4][:, (c % 4) * P:(c % 4 + 1) * P]
                            for t2 in range(tpc):
                                # start/stop are per 2KB PSUM zero region
                                # (whole bank): only the bank's first matmul
                                # starts, only its last stops.
                                nc.tensor.matmul(
                                    ps, lhsT=gbuf[:, c * tpc + t2, :],
                                    rhs=ohbuf[:, c * tpc + t2, :],
                                    start=(b == 0 and t2 == 0 and c % 4 == 0),
                                    stop=(b == cfg.nbuck - 1 and t2 == tpc - 1
                                          and (c % 4 == 3 or c == grp_sz - 1)))
                    if hooks is not None and grp in hooks:
                        hooks[grp]()
                    # drain the group's blocks
                    aggbf = []
                    for bk in range(nbank):
                        abf = wpool.tile([P, 4 * P], bf16, tag="aggbf", name="aggbf")
                        nc.vector.tensor_copy(abf[:], banks[bk][:])
                        aggbf.append(abf)
                    hgrp = None
                    if layer == 0:
                        hgrp = hgpool.tile([P, grp_sz, P], bf16, tag="hg",
                                           name="hgrp")
                    for c in range(grp_sz):
                        blk = grp * grp_sz + c
                        aggT = aggbf[c // 4][:, (c % 4) * P:(c % 4 + 1) * P]
                        ph = hpool.tile([P, P], f32, tag="ph", name="ph")
                        nc.tensor.matmul(ph[:], lhsT=aggT, rhs=Wl[:],
                                         start=True, stop=False)
                        # rank-1 bias: ph[slot, f] += (1/s_in)[slot] * b[f]
                        nc.tensor.matmul(ph[:],
                                         lhsT=rv_grp[0:1, c * P:(c + 1) * P],
                                         rhs=browl[:], start=False, stop=True)
                        if layer == 0:
                            # h1' = relu(alpha1 * ph), alpha1 = s_out*s_in
                            nc.scalar.activation(hgrp[:, c, :], ph[:], AF.Relu,
                                                 scale=alpha[:, blk:blk + 1])
                        else:
                            hslice = h2all[g][:, blk * P:(blk + 1) * P]
                            nc.scalar.activation(hslice, ph[:], AF.Identity,
                                                 scale=alpha[:, blk:blk + 1])
                            # sumsq operand straight from PSUM: (alpha*ph)^2
                            sqb = sqpool.tile([P, P], bf16, tag="sq", name="sqb")
                            nc.scalar.activation(sqb[:], ph[:], AF.Square,
                                                 scale=alpha[:, blk:blk + 1])
                            stp = st_psum[g]
                            # both stat slices share one zero region: start
                            # once (sum, blk 0), stop once (sumsq, last blk).
                            nc.tensor.matmul(stp[0:1, 0:P],
                                             lhsT=ones_bf[:, 0:1],
                                             rhs=hslice, start=(blk == 0),
                                             stop=False)
                            nc.tensor.matmul(stp[0:1, P:2 * P],
                                             lhsT=ones_bf[:, 0:1],
                                             rhs=sqb[:], start=False,
                                             stop=(blk == nblk - 1))
                    if layer == 0:
                        nc.sync.dma_start(
                            h1loc[g][grp * grp_sz * P:(grp + 1) * grp_sz * P, :]
                            .rearrange("(b p) f -> p b f", p=P),
                            hgrp[:])

            def finish_stats(g):
                """Drain the stats psum and push to HBM (frees the PSUM bank
                and lets the later collective issue without stalling)."""
                st_sb = wpool.tile([1, 2 * P], f32, tag="st", name="stsb")
                nc.vector.tensor_copy(st_sb[:], st_psum[g][:])
                nc.sync.dma_start(stloc[g][:], st_sb[:])

            def finish_collective(g):
                nc.gpsimd.collective_compute(
                    "AllGather", OP.bypass,
                    replica_groups=[list(range(NC))],
                    ins=[stloc[g][:].opt()], outs=[stall[g][:].opt()])

            def finish_graph(g):
                """Global stats + z-score + writeout for graph g."""
                st8 = wpool.tile([NC, 2 * P], f32, tag="st8", name="st8")
                nc.sync.dma_start(st8[:], stall[g][:])
                pred = hpool.tile([1, 2 * P], f32, tag="ph", name="pred")
                nc.tensor.matmul(pred[:], lhsT=ones[0:NC, 0:1], rhs=st8[:],
                                 start=True, stop=True)
                gs = wpool.tile([1, 2 * P], f32, tag="gs", name="gs")
                nc.vector.tensor_copy(gs[:], pred[:])
                ssum = gs[0:1, 0:P]
                ssq = gs[0:1, P:2 * P]
                nv = float(cfg.n)
                # var*(n-1) = ssq - ssum^2/n ; istd = rsqrt(var)
                m2 = wpool.tile([1, P], f32, tag="t0", name="m2")
                nc.vector.tensor_tensor(out=m2[:], in0=ssum, in1=ssum, op=OP.mult)
                nc.vector.tensor_scalar(out=m2[:], in0=m2[:], scalar1=-1.0 / nv,
                                        scalar2=None, op0=OP.mult)
                t1 = wpool.tile([1, P], f32, tag="t1", name="t1")
                nc.vector.tensor_tensor(out=t1[:], in0=ssq, in1=m2[:], op=OP.add)
                std = wpool.tile([1, P], f32, tag="std", name="std")
                nc.scalar.activation(std[:], t1[:], AF.Sqrt,
                                     scale=float(1.0 / (nv - 1.0)))
                istd = wpool.tile([1, P], f32, tag="istd", name="istd")
                nc.vector.reciprocal(istd[:], std[:])
                negms = wpool.tile([1, P], f32, tag="negms", name="negms")
                nc.vector.tensor_tensor(out=negms[:], in0=ssum, in1=istd[:],
                                        op=OP.mult)
                nc.vector.tensor_scalar(out=negms[:], in0=negms[:],
                                        scalar1=-1.0 / nv, scalar2=None,
                                        op0=OP.mult)
                # broadcast istd/negms to [P, P] bf16
                pA = hpool.tile([P, P], f32, tag="ph", name="pA")
                nc.tensor.matmul(pA[:], lhsT=ones[0:1, :], rhs=istd[:],
                                 start=True, stop=True)
                A = wpool.tile([P, P], bf16, tag="A", name="Abc")
                nc.vector.tensor_copy(A[:], pA[:])
                pC = hpool.tile([P, P], f32, tag="ph", name="pC")
                nc.tensor.matmul(pC[:], lhsT=ones[0:1, :], rhs=negms[:],
                                 start=True, stop=True)
                C = wpool.tile([P, P], bf16, tag="C", name="Cbc")
                nc.vector.tensor_copy(C[:], pC[:])
                # z in place over h2all, chunked so DMA-out overlaps the math
                h2 = h2all[g]
                nchunk = 4
                csz = nblk // nchunk
                for ci in range(nchunk):
                    h2v = h2[:, ci * csz * P:(ci + 1) * csz * P] \
                        .rearrange("p (b f) -> p b f", f=P)
                    Abc = A[:].rearrange("p (o f) -> p o f", o=1) \
                        .broadcast_to([P, csz, P])
                    Cbc = C[:].rearrange("p (o f) -> p o f", o=1) \
                        .broadcast_to([P, csz, P])
                    nc.vector.tensor_tensor(out=h2v, in0=h2v, in1=Abc, op=OP.mult)
                    nc.vector.tensor_tensor(out=h2v, in0=h2v, in1=Cbc, op=OP.add)
                    nc.sync.dma_start(
                        z_t[g][ci * csz * P:(ci + 1) * csz * P, :]
                        .rearrange("(b p) f -> p b f", p=P), h2v)

            # --- schedule ---
            # Collectives are issued just after the NEXT pass's first group of
            # gathers: by then their input DMAs have completed, so the shared
            # gpsimd sequencer never stalls the gather stream on a sem wait.
            def ag_h1(g):
                def go():
                    nc.gpsimd.collective_compute(
                        "AllGather", OP.bypass, replica_groups=[list(range(NC))],
                        ins=[h1loc[g][:].opt()], outs=[h1full[g][:].opt()])
                return go

            gather_layer(0, 0, feat_t[0][:], cfg.l1w, idx_sb=idx0_sb)
            gather_layer(1, 0, feat_t[1][:], cfg.l1w, hooks={0: ag_h1(0)})
            gather_layer(0, 1, h1full[0][:], cfg.l2w, hooks={0: ag_h1(1)})
            finish_stats(0)
            gather_layer(1, 1, h1full[1][:], cfg.l2w,
                         hooks={0: lambda: finish_collective(0),
                                3: lambda: finish_graph(0)})
            finish_stats(1)
            finish_collective(1)
            finish_graph(1)

    nc.finalize()
    return nc


# ---------------------------------------------------------------------------
# Entry point
# ---------------------------------------------------------------------------

def _prepare(inputs, cfg_kw=None):
    feat1 = np.asarray(inputs["feat1"], np.float32)
    feat2 = np.asarray(inputs["feat2"], np.float32)
    W1 = np.asarray(inputs["W1"], np.float32)
    b1 = np.asarray(inputs["b1"], np.float32).reshape(1, P)
    W2 = np.asarray(inputs["W2"], np.float32)
    b2 = np.asarray(inputs["b2"], np.float32).reshape(1, P)
    graphs = [(np.asarray(inputs["src1"], np.int64), np.asarray(inputs["dst1"], np.int64)),
              (np.asarray(inputs["src2"], np.int64), np.asarray(inputs["dst2"], np.int64))]

    n = feat1.shape[0]
    base = dict(n=n)
    if cfg_kw:
        base.update(cfg_kw)
    nblk = base.pop("nblk", 104)
    cfg = preps = None
    while True:
        cfg = Cfg(nblk=nblk, **base)
        preps = [_prep_graph(s, d, cfg) for s, d in graphs]
        if all(p is not None for p in preps):
            break
        nblk += cfg.ngrp
        assert nblk * P <= 4 * n, "block packing failed"

    iota = np.tile(np.arange(P, dtype=np.float32).astype(HDT)[None, :], (P, 1))
    # layer-1 gather tables: s_out-prescaled features
    f1b = (feat1 * preps[0]["s_out"][:, None]).astype(HDT)
    f2b = (feat2 * preps[1]["s_out"][:, None]).astype(HDT)
    in_maps = []
    for k in range(NC):
        m = dict(feat1=f1b, feat2=f2b, w1=W1.astype(HDT), w2=W2.astype(HDT),
                 b1=b1, b2=b2, iota=np.ascontiguousarray(iota))
        for g in range(2):
            ck = preps[g]["cores"][k]
            m[f"idxa{g+1}"] = ck["idxA"]
            m[f"idxb{g+1}"] = ck["idxB"]
            m[f"sid{g+1}"] = ck["sid"]
            m[f"al1{g+1}"] = ck["alpha1"]
            m[f"al2{g+1}"] = ck["alpha2"]
            m[f"rinv{g+1}"] = ck["rinv"]
        in_maps.append(m)
    return cfg, preps, in_maps


def _assemble(cfg, preps, results):
    outs = []
    for g in range(2):
        zp = np.concatenate([np.asarray(results[k][f"z{g+1}"])
                             for k in range(NC)], axis=0)
        outs.append(np.ascontiguousarray(zp[preps[g]["rowmap"]]).astype(np.float32))
    return outs[0], outs[1]


def _run(inputs, trace=False, tmpdir=None, cfg_kw=None):
    _ensure_env()
    from concourse import bass_utils

    cfg, preps, in_maps = _prepare(inputs, cfg_kw)
    nc = _build_program(cfg)

    kwargs = {}
    if trace:
        kwargs.update(trace=True)
        if tmpdir:
            kwargs.update(tmpdir=tmpdir)
    res = bass_utils.run_bass_kernel_spmd(nc, in_maps, core_ids=list(range(NC)),
                                          **kwargs)

    z1, z2 = _assemble(cfg, preps, res.results)
    return (z1, z2), res


def kernel(**inputs):
    (z1, z2), _ = _run(inputs)
    return (z1, z2)
